# revision 38
# baseline (speedup 1.0000x reference)
"""Distributed Trainium2 kernel for CustomMultiHeadAttentionStoich.

Head-parallel sharding (8 cores): core c = (batch b=c//4, head-group hg=c%4);
each core owns heads [4*hg, 4*hg+4) of its batch and runs them over ALL 2048
queries.  There is NO pre-attention collective: Q/K/V projections for the
local heads are computed from replicated inputs, so the only communication is
a per-query-chunk ReduceScatter of the (row-parallel) output-projection
partials, pipelined behind the next chunk's attention compute.

The stoichiometric bias alpha_pos*relu(clamp(fk-fq)) + alpha_neg*min(clamp,0)
is a rank-2x16 SVD feature expansion of the piecewise-linear kernel of
(fk - fq), injected as extra contraction rows of the scores matmul
(head_dim 64 leaves idle PE rows: zero extra PE cycles).

Scores are computed in [key, query] layout; softmax denominators come from a
fused ones-column of V through the AV matmul; normalization uses a rank-1 PE
broadcast of the reciprocals and a DVE multiply straight out of PSUM.
"""

import sys

sys.path.insert(0, "/opt/trn_rl_repo")

import numpy as np
import ml_dtypes

BF = ml_dtypes.bfloat16

B, T, D, H, DH = 2, 2048, 1024, 16, 64
NCORES = 8
HL = 4  # local heads per core
R = 16  # SVD rank per clamp-kernel half
AUG = 2 * R
CP = DH + AUG  # contraction rows for the scores matmul
NGRID = 1024  # SVD grid
KC = T // 128  # 16 key chunks
VG = DH + 1  # V columns per head incl. ones column
QC = 4  # query chunks (ReduceScatter granularity)
TQ = T // QC  # 512 queries per chunk

_state = {}


def _features():
    """Rank-R SVD features of rc(x,y)=clip(x-y,0,0.2) on [0,1]^2."""
    if "grid" not in _state:
        g = (np.arange(NGRID) + 0.5) / NGRID
        M = np.clip(g[:, None] - g[None, :], 0.0, 0.2)
        U, S, Vt = np.linalg.svd(M, full_matrices=False)
        sc = np.sqrt(S[:R] * NGRID)
        _state["grid"] = g
        _state["phi"] = (U[:, :R] * sc).astype(np.float64)  # [NGRID, R] phi_j(x)
        _state["psi"] = (Vt[:R].T * sc).astype(np.float64)  # [NGRID, R] psi_j(y)
    return _state["grid"], _state["phi"], _state["psi"]


def _ev(tab, x):
    g = _state["grid"]
    return np.stack([np.interp(x, g, tab[:, j]) for j in range(R)])


def _build():
    if "nc" in _state:
        return _state["nc"]

    import concourse.bass as bass
    import concourse.mybir as mybir
    import concourse.tile as tile
    from concourse import bacc

    dt = mybir.dt
    ts = bass.ts
    ds = bass.ds

    nc = bacc.Bacc(
        "TRN2",
        target_bir_lowering=False,
        debug=False,
        num_devices=NCORES,
    )

    # ---- kernel I/O (per-core shards; host pre-slices) ----
    xqT = nc.dram_tensor("xqT", [D, T], dt.bfloat16, kind="ExternalInput").ap()
    xkT = nc.dram_tensor("xkT", [D, T], dt.bfloat16, kind="ExternalInput").ap()
    xvT = nc.dram_tensor("xvT", [D, T], dt.bfloat16, kind="ExternalInput").ap()
    wqT = nc.dram_tensor("wqT", [D, HL * DH], dt.bfloat16, kind="ExternalInput").ap()
    wkT = nc.dram_tensor("wkT", [D, HL * DH], dt.bfloat16, kind="ExternalInput").ap()
    wvP = nc.dram_tensor("wvP", [D, HL * VG], dt.bfloat16, kind="ExternalInput").ap()
    woP = nc.dram_tensor("woP", [HL * DH, D], dt.bfloat16, kind="ExternalInput").ap()
    bqE = nc.dram_tensor("bq", [HL * DH, 1], dt.float32, kind="ExternalInput").ap()
    bkE = nc.dram_tensor("bk", [HL * DH, 1], dt.float32, kind="ExternalInput").ap()
    bvA = nc.dram_tensor("bvA", [1, HL * VG], dt.bfloat16, kind="ExternalInput").ap()
    boE = nc.dram_tensor("bo", [1, D], dt.bfloat16, kind="ExternalInput").ap()
    selE = nc.dram_tensor("sel2", [1, 2, 128], dt.bfloat16, kind="ExternalInput").ap()
    kfE = nc.dram_tensor("kfeat", [AUG, T], dt.bfloat16, kind="ExternalInput").ap()
    qfE = nc.dram_tensor("qfeat", [HL * AUG, T], dt.bfloat16, kind="ExternalInput").ap()
    outE = nc.dram_tensor("out", [QC * 128, D], dt.bfloat16, kind="ExternalOutput").ap()

    Exp = mybir.ActivationFunctionType.Exp
    Copy = mybir.ActivationFunctionType.Copy
    RG = [[0, 1, 2, 3], [4, 5, 6, 7]]
    Add = mybir.AluOpType.add

    with tile.TileContext(nc) as tc:
        with (
            tc.tile_pool(name="dram", bufs=1, space="DRAM") as dram,
            tc.tile_pool(name="consts", bufs=1) as consts,
            tc.tile_pool(name="ehat", bufs=3) as ep,
            tc.tile_pool(name="dsum", bufs=6) as dsp,
            tc.tile_pool(name="rsb", bufs=2) as rsbp,
            tc.tile_pool(name="aot", bufs=4) as aotp,
            tc.tile_pool(name="ystage", bufs=4) as ysp,
            tc.tile_pool(name="pp", bufs=2, space="PSUM") as pp,
            tc.tile_pool(name="psS", bufs=2, space="PSUM") as psS,
            tc.tile_pool(name="psAV", bufs=2, space="PSUM") as psAV,
        ):
            rs_in = [
                dram.tile([TQ, D], dt.bfloat16, tag=f"rs_in{q}", name=f"rs_in{q}")
                for q in range(QC)
            ]
            rs_out = [
                dram.tile([128, D], dt.bfloat16, tag=f"rs_out{q}", name=f"rs_out{q}")
                for q in range(QC)
            ]

            # ---- resident constants / activations ----
            xq_sb = consts.tile([128, 8, T], dt.bfloat16, tag="xq", name="xq")
            xk_sb = consts.tile([128, 8, T], dt.bfloat16, tag="xk", name="xk")
            xv_sb = consts.tile([128, 8, T], dt.bfloat16, tag="xv", name="xv")
            wq_sb = consts.tile([128, 8, HL * DH], dt.bfloat16, tag="wq", name="wq")
            wk_sb = consts.tile([128, 8, HL * DH], dt.bfloat16, tag="wk", name="wk")
            wv_sb = consts.tile([128, 8, HL * VG], dt.bfloat16, tag="wv", name="wv")
            wo_sb = consts.tile([128, 2, D], dt.bfloat16, tag="wo", name="wo")
            bq_sb = consts.tile([128, 2, 1], dt.float32, tag="bq", name="bq")
            bk_sb = consts.tile([128, 2, 1], dt.float32, tag="bk", name="bk")
            bv_sb = consts.tile([1, HL * VG], dt.bfloat16, tag="bv", name="bv")
            bo_sb = consts.tile([1, D], dt.bfloat16, tag="bo", name="bo")
            ones_sb = consts.tile([1, 128], dt.bfloat16, tag="ones", name="ones")
            sel2 = consts.tile([1, 2, 128], dt.bfloat16, tag="sel2", name="sel2")

            kat = [
                consts.tile([CP, T], dt.bfloat16, tag=f"kat{h}", name=f"kat{h}")
                for h in range(HL)
            ]
            qat = [
                consts.tile([CP, T], dt.bfloat16, tag=f"qat{h}", name=f"qat{h}")
                for h in range(HL)
            ]
            vtl = consts.tile([128, KC, HL * VG], dt.bfloat16, tag="vtl", name="vtl")

            # ---- DMA loads (weights first, then per-contraction-chunk x) ----
            nc.sync.dma_start(
                out=wk_sb, in_=wkT.rearrange("(a p) m -> p a m", p=128)
            )
            nc.sync.dma_start(
                out=wv_sb, in_=wvP.rearrange("(a p) m -> p a m", p=128)
            )
            nc.sync.dma_start(
                out=wq_sb, in_=wqT.rearrange("(a p) m -> p a m", p=128)
            )
            nc.sync.dma_start(out=wo_sb, in_=woP.rearrange("(g p) m -> p g m", p=128))
            nc.sync.dma_start(out=bq_sb, in_=bqE.rearrange("(g p) o -> p g o", p=128))
            nc.sync.dma_start(out=bk_sb, in_=bkE.rearrange("(g p) o -> p g o", p=128))
            nc.sync.dma_start(out=bv_sb, in_=bvA)
            nc.sync.dma_start(out=bo_sb, in_=boE)
            for a in range(8):
                nc.sync.dma_start(out=xk_sb[:, a, :], in_=xkT[ds(128 * a, 128), :])
            for a in range(8):
                nc.sync.dma_start(out=xv_sb[:, a, :], in_=xvT[ds(128 * a, 128), :])
            for a in range(8):
                nc.sync.dma_start(out=xq_sb[:, a, :], in_=xqT[ds(128 * a, 128), :])
            for h in range(HL):
                nc.sync.dma_start(out=kat[h][DH:CP, :], in_=kfE)
                nc.sync.dma_start(out=qat[h][DH:CP, :], in_=qfE[ds(AUG * h, AUG), :])
            nc.vector.memset(ones_sb, 1.0)
            nc.sync.dma_start(out=sel2, in_=selE)

            # ---- warmup collective: absorb first-RS setup cost early ----
            wu_in = dram.tile([4, 128], dt.bfloat16, tag="wu_in", name="wu_in")
            wu_out = dram.tile([1, 128], dt.bfloat16, tag="wu_out", name="wu_out")
            wu_sb = consts.tile([4, 128], dt.bfloat16, tag="wu_sb", name="wu_sb")
            nc.vector.memset(wu_sb, 0.0)
            nc.sync.dma_start(out=wu_in, in_=wu_sb)
            nc.gpsimd.collective_compute(
                "ReduceScatter",
                Add,
                ins=[wu_in.opt()],
                outs=[wu_out.opt()],
                replica_groups=RG,
            )

            # ---- K^T projection: kat[h][0:64, :] = Wk_h @ xk^T + bk ----
            for g in range(2):
                for tc_i in range(4):
                    ps = pp.tile([128, 512], dt.float32, tag="mm", name="mmk")
                    for kc in range(8):
                        nc.tensor.matmul(
                            ps,
                            lhsT=wk_sb[:, kc, ts(g, 128)],
                            rhs=xk_sb[:, kc, ts(tc_i, 512)],
                            start=(kc == 0),
                            stop=(kc == 7),
                        )
                    for j in range(2):
                        nc.vector.tensor_scalar_add(
                            kat[2 * g + j][0:DH, ts(tc_i, 512)],
                            ps[ds(DH * j, DH), :],
                            bk_sb[ds(DH * j, DH), g, :],
                        )

            # ---- V projection (ones-column fused): vtl[:, t, :] ----
            for tc_i in range(KC):
                ps = pp.tile([128, HL * VG], dt.float32, tag="mm", name="mmv")
                for kc in range(8):
                    nc.tensor.matmul(
                        ps,
                        lhsT=xv_sb[:, kc, ts(tc_i, 128)],
                        rhs=wv_sb[:, kc, :],
                        start=(kc == 0),
                        stop=False,
                    )
                nc.tensor.matmul(
                    ps,
                    lhsT=ones_sb[:, :],
                    rhs=bv_sb[:, :],
                    start=False,
                    stop=True,
                )
                nc.vector.tensor_copy(vtl[:, tc_i, :], ps)

            # ---- Q projection: qat[h][0:64, :] = gs * Wq_h @ xq^T + bq ----
            for g in range(2):
                for tc_i in range(4):
                    ps = pp.tile([128, 512], dt.float32, tag="mm", name="mmq")
                    for kc in range(8):
                        nc.tensor.matmul(
                            ps,
                            lhsT=wq_sb[:, kc, ts(g, 128)],
                            rhs=xq_sb[:, kc, ts(tc_i, 512)],
                            start=(kc == 0),
                            stop=(kc == 7),
                        )
                    for j in range(2):
                        nc.vector.tensor_scalar_add(
                            qat[2 * g + j][0:DH, ts(tc_i, 512)],
                            ps[ds(DH * j, DH), :],
                            bq_sb[ds(DH * j, DH), g, :],
                        )

            # ---- attention + row-parallel out-projection, per query chunk ----
            # The PE queue is in-order, so AV(kp) is emitted AFTER S(kp+1):
            # the PE streams scores while the scalar engine exps the previous
            # pair of key chunks, and AV picks up the exp'd tile one step
            # later. Scalar runs a pure-exp stream (all small copies on DVE).
            KP = KC // 2
            outproj_due = []  # deferred out-projection groups of the prev qc

            def emit_outproj_group(pqc, p_aot, qs, mc):
                ps_y = pp.tile([128, 512], dt.float32, tag="mm", name="mmy")
                for g in range(2):
                    nc.tensor.matmul(
                        ps_y,
                        lhsT=p_aot[g][:, ts(qs, 128)],
                        rhs=wo_sb[:, g, ts(mc, 512)],
                        start=(g == 0),
                        stop=False,
                    )
                nc.tensor.matmul(
                    ps_y,
                    lhsT=ones_sb,
                    rhs=bo_sb[:, ts(mc, 512)],
                    start=False,
                    stop=True,
                )
                ystg = ysp.tile([128, 512], dt.bfloat16, tag="ystage", name="ystage")
                nc.vector.tensor_copy(ystg, ps_y)
                nc.sync.dma_start(
                    out=rs_in[pqc][ds(128 * qs, 128), ts(mc, 512)],
                    in_=ystg,
                )

            def emit_collective(pqc):
                nc.gpsimd.collective_compute(
                    "ReduceScatter",
                    Add,
                    ins=[rs_in[pqc].opt()],
                    outs=[rs_out[pqc].opt()],
                    replica_groups=RG,
                )

            for qc in range(QC):
                aot_pair = [
                    aotp.tile([128, TQ], dt.bfloat16, tag=f"aot{g}", name=f"aot{g}")
                    for g in range(2)
                ]
                av_by = {}
                rbf_by = {}
                pending = []

                def flush_av():
                    h, kp, eh, ps_av = pending.pop(0)
                    for u in range(2):
                        nc.tensor.matmul(
                            ps_av,
                            lhsT=vtl[:, 2 * kp + u, ds(VG * h, VG)],
                            rhs=eh[:, u, :],
                            start=(kp == 0 and u == 0),
                            stop=(kp == KP - 1 and u == 1),
                        )
                    if kp == KP - 1:
                        dsum = dsp.tile([1, TQ], dt.float32, tag="dsum", name="dsum")
                        rcp = dsp.tile([1, TQ], dt.float32, tag="rcp", name="rcp")
                        rbf = dsp.tile(
                            [1, TQ], dt.bfloat16, tag="rcpbf", name="rcpbf"
                        )
                        nc.vector.tensor_copy(dsum, ps_av[ds(DH, 1), :])
                        nc.vector.reciprocal_approx_fast(rcp, dsum)
                        nc.vector.tensor_copy(rbf, rcp)
                        rbf_by[h] = rbf

                def norm_pair(g):
                    ps_r = pp.tile([128, TQ], dt.float32, tag="mm", name="mmr")
                    for j in range(2):
                        nc.tensor.matmul(
                            ps_r,
                            lhsT=sel2[:, j, :],
                            rhs=rbf_by[2 * g + j],
                            start=(j == 0),
                            stop=(j == 1),
                        )
                    r_sb = rsbp.tile([128, TQ], dt.bfloat16, tag="rsb", name="rsb")
                    nc.vector.tensor_copy(r_sb, ps_r)
                    for j in range(2):
                        nc.vector.tensor_mul(
                            aot_pair[g][ds(DH * j, DH), :],
                            av_by[2 * g + j][0:DH, :],
                            r_sb[ds(DH * j, DH), :],
                        )

                norm_due = None
                for h in range(HL):
                    ps_av = psAV.tile([VG, TQ], dt.float32, tag="av", name="av")
                    av_by[h] = ps_av
                    for kp in range(KP):
                        ps_s = psS.tile([128, 2, TQ], dt.float32, tag="s", name="s")
                        eh = ep.tile([128, 2, TQ], dt.bfloat16, tag="ehat", name="eh")
                        for u in range(2):
                            nc.tensor.matmul(
                                ps_s[:, u, :],
                                lhsT=kat[h][:, ts(2 * kp + u, 128)],
                                rhs=qat[h][:, ts(qc, TQ)],
                                start=True,
                                stop=True,
                            )
                        nc.scalar.activation(eh, ps_s, Exp)
                        if norm_due is not None:
                            # normalization of the finished pair goes into the
                            # PE queue BEFORE this head's first AV (which
                            # recycles that pair's PSUM buffer)
                            norm_pair(norm_due)
                            norm_due = None
                        if outproj_due:
                            # spread the previous chunk's out-projection into
                            # this chunk's PE stream instead of serializing it
                            emit_outproj_group(*outproj_due.pop(0))
                            if not outproj_due:
                                emit_collective(qc - 1)
                        if pending:
                            flush_av()
                        pending.append((h, kp, eh, ps_av))
                    if h % 2 == 1:
                        while pending:
                            flush_av()
                        norm_due = h // 2
                if norm_due is not None:
                    norm_pair(norm_due)
                    norm_due = None
                outproj_due = [
                    (qc, aot_pair, qs, mc) for qs in range(4) for mc in range(2)
                ]
            while outproj_due:
                emit_outproj_group(*outproj_due.pop(0))
            emit_collective(QC - 1)

            for qc in range(QC):
                nc.sync.dma_start(
                    out=outE[ds(128 * qc, 128), :], in_=rs_out[qc][:, :]
                )

    nc.compile()
    _state["nc"] = nc
    return nc


def _make_in_maps(inputs):
    _features()
    gs = float(np.float32(inputs["gamma"])) * DH ** -0.5
    delta = float(np.float32(inputs["delta"]))
    ap_ = np.asarray(inputs["alpha_pos"], np.float64)
    an_ = np.asarray(inputs["alpha_neg"], np.float64)

    wqT_full = np.asarray(inputs["Wq"], np.float64).T * gs
    bq_full = (np.asarray(inputs["bq"], np.float64) * gs).astype(np.float32)
    wkT_full = np.ascontiguousarray(np.asarray(inputs["Wk"]).T)
    bk_full = np.asarray(inputs["bk"], np.float32)
    wvT_full = np.asarray(inputs["Wv"], np.float64).T
    bv_full = np.asarray(inputs["bv"], np.float64)
    woT_full = np.ascontiguousarray(np.asarray(inputs["Wo"]).T).astype(BF)
    bo_full = np.asarray(inputs["bo"], np.float32)[None, :].astype(BF)

    phi, psi = _state["phi"], _state["psi"]
    frac = np.asarray(inputs["frac"], np.float64)

    sel2 = np.zeros((1, 2, 128), np.float64)
    sel2[0, 0, :DH] = 1.0
    sel2[0, 1, DH:] = 1.0
    sel2 = sel2.astype(BF)

    # per-batch feature tables (evaluated at every position)
    kfeat_b, phi_b, psi_b = [], [], []
    for b in range(B):
        fb = frac[b]
        pf, sf = _ev(phi, fb), _ev(psi, fb)
        phi_b.append(pf)
        psi_b.append(sf)
        kfeat_b.append(np.concatenate([pf, sf], 0).astype(BF))

    xqT_b = [
        np.ascontiguousarray(np.asarray(inputs["query"])[b].T).astype(BF)
        for b in range(B)
    ]
    xkT_b = [
        np.ascontiguousarray(np.asarray(inputs["key"])[b].T).astype(BF)
        for b in range(B)
    ]
    xvT_b = [
        np.ascontiguousarray(np.asarray(inputs["value"])[b].T).astype(BF)
        for b in range(B)
    ]

    in_maps = []
    for c in range(NCORES):
        b, hg = c // 4, c % 4
        h0 = HL * hg  # first global head

        wvP = np.zeros((D, HL * VG), np.float64)
        bvA = np.zeros((1, HL * VG), np.float64)
        for j in range(HL):
            wvP[:, VG * j : VG * j + DH] = wvT_full[:, DH * (h0 + j) : DH * (h0 + j) + DH]
            bvA[0, VG * j : VG * j + DH] = bv_full[DH * (h0 + j) : DH * (h0 + j) + DH]
            bvA[0, VG * j + DH] = 1.0

        qfeat = np.zeros((HL * AUG, T), np.float64)
        for j in range(HL):
            h = h0 + j
            a_h = delta * ap_[h] / NGRID
            b_h = -delta * an_[h] / NGRID
            qfeat[AUG * j : AUG * j + R] = a_h * psi_b[b]
            qfeat[AUG * j + R : AUG * (j + 1)] = b_h * phi_b[b]

        in_maps.append(
            {
                "xqT": xqT_b[b],
                "xkT": xkT_b[b],
                "xvT": xvT_b[b],
                "wqT": np.ascontiguousarray(
                    wqT_full[:, DH * h0 : DH * h0 + HL * DH]
                ).astype(BF),
                "wkT": np.ascontiguousarray(
                    wkT_full[:, DH * h0 : DH * h0 + HL * DH]
                ).astype(BF),
                "wvP": wvP.astype(BF),
                "woP": np.ascontiguousarray(
                    woT_full[DH * h0 : DH * h0 + HL * DH, :]
                ),
                "bq": np.ascontiguousarray(bq_full[DH * h0 : DH * h0 + HL * DH])[
                    :, None
                ],
                "bk": np.ascontiguousarray(bk_full[DH * h0 : DH * h0 + HL * DH])[
                    :, None
                ],
                "bvA": bvA.astype(BF),
                "sel2": sel2,
                "bo": bo_full if hg == 0 else np.zeros_like(bo_full),
                "kfeat": kfeat_b[b],
                "qfeat": qfeat.astype(BF),
            }
        )
    return in_maps


def _run(inputs, trace=False, **kw):
    from concourse.bass_utils import run_bass_kernel_spmd

    nc = _build()
    in_maps = _make_in_maps(inputs)
    res = run_bass_kernel_spmd(
        nc, in_maps, core_ids=list(range(NCORES)), trace=trace, **kw
    )
    out = np.zeros((B, T, D), np.float32)
    for c in range(NCORES):
        b, p = c // 4, c % 4
        o = np.asarray(res.results[c]["out"], np.float32)  # [QC*128, D]
        for qc in range(QC):
            out[b, TQ * qc + 128 * p : TQ * qc + 128 * p + 128] = o[
                128 * qc : 128 * qc + 128
            ]
    return out, res


def kernel(**inputs):
    out, _ = _run(inputs)
    return out


# revision 43
# speedup vs baseline: 1.1013x; 1.1013x over previous
"""Distributed Trainium2 kernel for CustomMultiHeadAttentionStoich.

Head-parallel sharding (8 cores): core c = (batch b=c//4, head-group hg=c%4);
each core owns heads [4*hg, 4*hg+4) of its batch and runs them over ALL 2048
queries.  There is NO pre-attention collective: Q/K/V projections for the
local heads are computed from replicated inputs, so the only communication is
a per-query-chunk ReduceScatter of the (row-parallel) output-projection
partials, pipelined behind the next chunk's attention compute.

The stoichiometric bias alpha_pos*relu(clamp(fk-fq)) + alpha_neg*min(clamp,0)
is a rank-2x16 SVD feature expansion of the piecewise-linear kernel of
(fk - fq), injected as extra contraction rows of the scores matmul
(head_dim 64 leaves idle PE rows: zero extra PE cycles).

Scores are computed in [key, query] layout; softmax denominators come from a
fused ones-column of V through the AV matmul; normalization uses a rank-1 PE
broadcast of the reciprocals and a DVE multiply straight out of PSUM.
"""

import sys

sys.path.insert(0, "/opt/trn_rl_repo")

import numpy as np
import ml_dtypes

BF = ml_dtypes.bfloat16

B, T, D, H, DH = 2, 2048, 1024, 16, 64
NCORES = 8
HL = 4  # local heads per core
R = 16  # SVD rank per clamp-kernel half
AUG = 2 * R
CP = DH + AUG  # contraction rows for the scores matmul
NGRID = 1024  # SVD grid
KC = T // 128  # 16 key chunks
VG = DH + 1  # V columns per head incl. ones column
QC = 4  # query chunks (ReduceScatter granularity)
TQ = T // QC  # 512 queries per chunk

_state = {}


def _features():
    """Rank-R SVD features of rc(x,y)=clip(x-y,0,0.2) on [0,1]^2."""
    if "grid" not in _state:
        g = (np.arange(NGRID) + 0.5) / NGRID
        M = np.clip(g[:, None] - g[None, :], 0.0, 0.2)
        U, S, Vt = np.linalg.svd(M, full_matrices=False)
        sc = np.sqrt(S[:R] * NGRID)
        _state["grid"] = g
        _state["phi"] = (U[:, :R] * sc).astype(np.float64)  # [NGRID, R] phi_j(x)
        _state["psi"] = (Vt[:R].T * sc).astype(np.float64)  # [NGRID, R] psi_j(y)
    return _state["grid"], _state["phi"], _state["psi"]


def _ev(tab, x):
    g = _state["grid"]
    return np.stack([np.interp(x, g, tab[:, j]) for j in range(R)])


def _build(bias_free=False):
    if "nc" in _state:
        return _state["nc"]

    import concourse.bass as bass
    import concourse.mybir as mybir
    import concourse.tile as tile
    from concourse import bacc

    dt = mybir.dt
    ts = bass.ts
    ds = bass.ds

    nc = bacc.Bacc(
        "TRN2",
        target_bir_lowering=False,
        debug=False,
        num_devices=NCORES,
    )

    # ---- kernel I/O (per-core shards; host pre-slices) ----
    xqT = nc.dram_tensor("xqT", [D, T], dt.bfloat16, kind="ExternalInput").ap()
    xkT = nc.dram_tensor("xkT", [D, T], dt.bfloat16, kind="ExternalInput").ap()
    xvT = nc.dram_tensor("xvT", [D, T], dt.bfloat16, kind="ExternalInput").ap()
    wqT = nc.dram_tensor("wqT", [D, HL * DH], dt.bfloat16, kind="ExternalInput").ap()
    wkT = nc.dram_tensor("wkT", [D, HL * DH], dt.bfloat16, kind="ExternalInput").ap()
    wvP = nc.dram_tensor("wvP", [D, HL * VG], dt.bfloat16, kind="ExternalInput").ap()
    woP = nc.dram_tensor("woP", [HL * DH, D], dt.bfloat16, kind="ExternalInput").ap()
    bqE = nc.dram_tensor("bq", [HL * DH, 1], dt.float32, kind="ExternalInput").ap()
    bkE = nc.dram_tensor("bk", [HL * DH, 1], dt.float32, kind="ExternalInput").ap()
    bvA = nc.dram_tensor("bvA", [1, HL * VG], dt.bfloat16, kind="ExternalInput").ap()
    boE = nc.dram_tensor("bo", [1, D], dt.bfloat16, kind="ExternalInput").ap()
    selE = nc.dram_tensor("sel2", [1, 2, 128], dt.bfloat16, kind="ExternalInput").ap()
    kfE = nc.dram_tensor("kfeat", [AUG, T], dt.bfloat16, kind="ExternalInput").ap()
    qfE = nc.dram_tensor("qfeat", [HL * AUG, T], dt.bfloat16, kind="ExternalInput").ap()
    outE = nc.dram_tensor("out", [QC * 128, D], dt.bfloat16, kind="ExternalOutput").ap()

    Exp = mybir.ActivationFunctionType.Exp
    Copy = mybir.ActivationFunctionType.Copy
    RG = [[0, 1, 2, 3], [4, 5, 6, 7]]
    Add = mybir.AluOpType.add

    with tile.TileContext(nc) as tc:
        with (
            tc.tile_pool(name="dram", bufs=1, space="DRAM") as dram,
            tc.tile_pool(name="consts", bufs=1) as consts,
            tc.tile_pool(name="ehat", bufs=3) as ep,
            tc.tile_pool(name="dsum", bufs=6) as dsp,
            tc.tile_pool(name="rsb", bufs=2) as rsbp,
            tc.tile_pool(name="aot", bufs=4) as aotp,
            tc.tile_pool(name="ystage", bufs=4) as ysp,
            tc.tile_pool(name="pp", bufs=2, space="PSUM") as pp,
            tc.tile_pool(name="psS", bufs=2, space="PSUM") as psS,
            tc.tile_pool(name="psAV", bufs=2, space="PSUM") as psAV,
        ):
            rs_in = [
                dram.tile([TQ, D], dt.bfloat16, tag=f"rs_in{q}", name=f"rs_in{q}")
                for q in range(QC)
            ]
            rs_out = [
                dram.tile([128, D], dt.bfloat16, tag=f"rs_out{q}", name=f"rs_out{q}")
                for q in range(QC)
            ]

            # ---- resident constants / activations ----
            xq_sb = consts.tile([128, 8, T], dt.bfloat16, tag="xq", name="xq")
            xk_sb = consts.tile([128, 8, T], dt.bfloat16, tag="xk", name="xk")
            xv_sb = consts.tile([128, 8, T], dt.bfloat16, tag="xv", name="xv")
            wq_sb = consts.tile([128, 8, HL * DH], dt.bfloat16, tag="wq", name="wq")
            wk_sb = consts.tile([128, 8, HL * DH], dt.bfloat16, tag="wk", name="wk")
            wv_sb = consts.tile([128, 8, HL * VG], dt.bfloat16, tag="wv", name="wv")
            wo_sb = consts.tile([128, 2, D], dt.bfloat16, tag="wo", name="wo")
            bq_sb = consts.tile([128, 2, 1], dt.float32, tag="bq", name="bq")
            bk_sb = consts.tile([128, 2, 1], dt.float32, tag="bk", name="bk")
            bv_sb = consts.tile([1, HL * VG], dt.bfloat16, tag="bv", name="bv")
            bo_sb = consts.tile([1, D], dt.bfloat16, tag="bo", name="bo")
            ones_sb = consts.tile([1, 128], dt.bfloat16, tag="ones", name="ones")
            sel2 = consts.tile([1, 2, 128], dt.bfloat16, tag="sel2", name="sel2")

            kat = [
                consts.tile([CP, T], dt.bfloat16, tag=f"kat{h}", name=f"kat{h}")
                for h in range(HL)
            ]
            qat = [
                consts.tile([CP, T], dt.bfloat16, tag=f"qat{h}", name=f"qat{h}")
                for h in range(HL)
            ]
            vtl = consts.tile([128, KC, HL * VG], dt.bfloat16, tag="vtl", name="vtl")

            # ---- DMA loads (weights first, then per-contraction-chunk x) ----
            nc.sync.dma_start(
                out=wk_sb, in_=wkT.rearrange("(a p) m -> p a m", p=128)
            )
            nc.sync.dma_start(
                out=wv_sb, in_=wvP.rearrange("(a p) m -> p a m", p=128)
            )
            nc.sync.dma_start(
                out=wq_sb, in_=wqT.rearrange("(a p) m -> p a m", p=128)
            )
            nc.sync.dma_start(out=wo_sb, in_=woP.rearrange("(g p) m -> p g m", p=128))
            nc.sync.dma_start(out=bq_sb, in_=bqE.rearrange("(g p) o -> p g o", p=128))
            nc.sync.dma_start(out=bk_sb, in_=bkE.rearrange("(g p) o -> p g o", p=128))
            nc.sync.dma_start(out=bv_sb, in_=bvA)
            nc.sync.dma_start(out=bo_sb, in_=boE)
            for a in range(8):
                nc.sync.dma_start(out=xk_sb[:, a, :], in_=xkT[ds(128 * a, 128), :])
            for a in range(8):
                nc.sync.dma_start(out=xv_sb[:, a, :], in_=xvT[ds(128 * a, 128), :])
            for a in range(8):
                nc.sync.dma_start(out=xq_sb[:, a, :], in_=xqT[ds(128 * a, 128), :])
            for h in range(HL):
                nc.sync.dma_start(out=kat[h][DH:CP, :], in_=kfE)
                nc.sync.dma_start(out=qat[h][DH:CP, :], in_=qfE[ds(AUG * h, AUG), :])
            nc.vector.memset(ones_sb, 1.0)
            nc.sync.dma_start(out=sel2, in_=selE)

            # ---- warmup collective: absorb first-RS setup cost early ----
            wu_in = dram.tile([4, 128], dt.bfloat16, tag="wu_in", name="wu_in")
            wu_out = dram.tile([1, 128], dt.bfloat16, tag="wu_out", name="wu_out")
            wu_sb = consts.tile([4, 128], dt.bfloat16, tag="wu_sb", name="wu_sb")
            nc.vector.memset(wu_sb, 0.0)
            nc.sync.dma_start(out=wu_in, in_=wu_sb)
            nc.gpsimd.collective_compute(
                "ReduceScatter",
                Add,
                ins=[wu_in.opt()],
                outs=[wu_out.opt()],
                replica_groups=RG,
            )

            # ---- K^T projection: kat[h][0:64, :] = Wk_h @ xk^T + bk ----
            for g in range(2):
                for tc_i in range(4):
                    ps = pp.tile([128, 512], dt.float32, tag="mm", name="mmk")
                    for kc in range(8):
                        nc.tensor.matmul(
                            ps,
                            lhsT=wk_sb[:, kc, ts(g, 128)],
                            rhs=xk_sb[:, kc, ts(tc_i, 512)],
                            start=(kc == 0),
                            stop=(kc == 7),
                        )
                    for j in range(2):
                        nc.vector.tensor_scalar_add(
                            kat[2 * g + j][0:DH, ts(tc_i, 512)],
                            ps[ds(DH * j, DH), :],
                            bk_sb[ds(DH * j, DH), g, :],
                        )

            # ---- V projection (ones-column fused): vtl[:, t, :] ----
            for tc_i in range(KC):
                ps = pp.tile([128, HL * VG], dt.float32, tag="mm", name="mmv")
                for kc in range(8):
                    nc.tensor.matmul(
                        ps,
                        lhsT=xv_sb[:, kc, ts(tc_i, 128)],
                        rhs=wv_sb[:, kc, :],
                        start=(kc == 0),
                        stop=False,
                    )
                nc.tensor.matmul(
                    ps,
                    lhsT=ones_sb[:, :],
                    rhs=bv_sb[:, :],
                    start=False,
                    stop=True,
                )
                nc.vector.tensor_copy(vtl[:, tc_i, :], ps)

            # ---- Q projection: qat[h][0:64, :] = gs * Wq_h @ xq^T + bq ----
            for g in range(2):
                for tc_i in range(4):
                    ps = pp.tile([128, 512], dt.float32, tag="mm", name="mmq")
                    for kc in range(8):
                        nc.tensor.matmul(
                            ps,
                            lhsT=wq_sb[:, kc, ts(g, 128)],
                            rhs=xq_sb[:, kc, ts(tc_i, 512)],
                            start=(kc == 0),
                            stop=(kc == 7),
                        )
                    for j in range(2):
                        nc.vector.tensor_scalar_add(
                            qat[2 * g + j][0:DH, ts(tc_i, 512)],
                            ps[ds(DH * j, DH), :],
                            bq_sb[ds(DH * j, DH), g, :],
                        )

            # ---- attention + row-parallel out-projection, per query chunk ----
            # The PE queue is in-order, so AV(kp) is emitted AFTER S(kp+1):
            # the PE streams scores while the scalar engine exps the previous
            # pair of key chunks, and AV picks up the exp'd tile one step
            # later. Scalar runs a pure-exp stream (all small copies on DVE).
            KP = KC // 2

            def emit_outproj_group(pqc, p_aot, qs, mc):
                ps_y = pp.tile([128, 512], dt.float32, tag="mm", name="mmy")
                for g in range(2):
                    nc.tensor.matmul(
                        ps_y,
                        lhsT=p_aot[g][:, ts(qs, 128)],
                        rhs=wo_sb[:, g, ts(mc, 512)],
                        start=(g == 0),
                        stop=(g == 1 and bias_free),
                    )
                if not bias_free:
                    nc.tensor.matmul(
                        ps_y,
                        lhsT=ones_sb,
                        rhs=bo_sb[:, ts(mc, 512)],
                        start=False,
                        stop=True,
                    )
                ystg = ysp.tile([128, 512], dt.bfloat16, tag="ystage", name="ystage")
                nc.vector.tensor_copy(ystg, ps_y)
                nc.sync.dma_start(
                    out=rs_in[pqc][ds(128 * qs, 128), ts(mc, 512)],
                    in_=ystg,
                )

            def emit_collective(pqc):
                nc.gpsimd.collective_compute(
                    "ReduceScatter",
                    Add,
                    ins=[rs_in[pqc].opt()],
                    outs=[rs_out[pqc].opt()],
                    replica_groups=RG,
                )

            for qc in range(QC):
                aot_pair = [
                    aotp.tile([128, TQ], dt.bfloat16, tag=f"aot{g}", name=f"aot{g}")
                    for g in range(2)
                ]
                av_by = {}
                rbf_by = {}
                pending = []

                def flush_av():
                    h, kp, eh, ps_av = pending.pop(0)
                    for u in range(2):
                        nc.tensor.matmul(
                            ps_av,
                            lhsT=vtl[:, 2 * kp + u, ds(VG * h, VG)],
                            rhs=eh[:, u, :],
                            start=(kp == 0 and u == 0),
                            stop=(kp == KP - 1 and u == 1),
                        )
                    if kp == KP - 1:
                        dsum = dsp.tile([1, TQ], dt.float32, tag="dsum", name="dsum")
                        rcp = dsp.tile([1, TQ], dt.float32, tag="rcp", name="rcp")
                        rbf = dsp.tile(
                            [1, TQ], dt.bfloat16, tag="rcpbf", name="rcpbf"
                        )
                        nc.vector.tensor_copy(dsum, ps_av[ds(DH, 1), :])
                        nc.vector.reciprocal_approx_fast(rcp, dsum)
                        nc.vector.tensor_copy(rbf, rcp)
                        rbf_by[h] = rbf

                def norm_pair(g):
                    ps_r = pp.tile([128, TQ], dt.float32, tag="mm", name="mmr")
                    for j in range(2):
                        nc.tensor.matmul(
                            ps_r,
                            lhsT=sel2[:, j, :],
                            rhs=rbf_by[2 * g + j],
                            start=(j == 0),
                            stop=(j == 1),
                        )
                    r_sb = rsbp.tile([128, TQ], dt.bfloat16, tag="rsb", name="rsb")
                    nc.vector.tensor_copy(r_sb, ps_r)
                    for j in range(2):
                        nc.vector.tensor_mul(
                            aot_pair[g][ds(DH * j, DH), :],
                            av_by[2 * g + j][0:DH, :],
                            r_sb[ds(DH * j, DH), :],
                        )

                norm_due = None
                for h in range(HL):
                    ps_av = psAV.tile([VG, TQ], dt.float32, tag="av", name="av")
                    av_by[h] = ps_av
                    for kp in range(KP):
                        ps_s = psS.tile([128, 2, TQ], dt.float32, tag="s", name="s")
                        eh = ep.tile([128, 2, TQ], dt.bfloat16, tag="ehat", name="eh")
                        for u in range(2):
                            nc.tensor.matmul(
                                ps_s[:, u, :],
                                lhsT=kat[h][:, ts(2 * kp + u, 128)],
                                rhs=qat[h][:, ts(qc, TQ)],
                                start=True,
                                stop=True,
                            )
                        nc.scalar.activation(eh, ps_s, Exp)
                        if norm_due is not None:
                            # normalization of the finished pair goes into the
                            # PE queue BEFORE this head's first AV (which
                            # recycles that pair's PSUM buffer)
                            norm_pair(norm_due)
                            norm_due = None
                        if pending:
                            flush_av()
                        pending.append((h, kp, eh, ps_av))
                    if h % 2 == 1:
                        while pending:
                            flush_av()
                        norm_due = h // 2
                if norm_due is not None:
                    norm_pair(norm_due)
                    norm_due = None
                for qs in range(4):
                    for mc in range(2):
                        emit_outproj_group(qc, aot_pair, qs, mc)
                emit_collective(qc)

            for qc in range(QC):
                nc.sync.dma_start(
                    out=outE[ds(128 * qc, 128), :], in_=rs_out[qc][:, :]
                )

    nc.compile()
    _state["nc"] = nc
    return nc


def _make_in_maps(inputs):
    _features()
    gs = float(np.float32(inputs["gamma"])) * DH ** -0.5
    delta = float(np.float32(inputs["delta"]))
    ap_ = np.asarray(inputs["alpha_pos"], np.float64)
    an_ = np.asarray(inputs["alpha_neg"], np.float64)

    wqT_full = np.asarray(inputs["Wq"], np.float64).T * gs
    bq_full = (np.asarray(inputs["bq"], np.float64) * gs).astype(np.float32)
    wkT_full = np.ascontiguousarray(np.asarray(inputs["Wk"]).T)
    bk_full = np.asarray(inputs["bk"], np.float32)
    wvT_full = np.asarray(inputs["Wv"], np.float64).T
    bv_full = np.asarray(inputs["bv"], np.float64)
    woT_full = np.ascontiguousarray(np.asarray(inputs["Wo"]).T).astype(BF)
    bo_full = np.asarray(inputs["bo"], np.float32)[None, :].astype(BF)

    phi, psi = _state["phi"], _state["psi"]
    frac = np.asarray(inputs["frac"], np.float64)

    sel2 = np.zeros((1, 2, 128), np.float64)
    sel2[0, 0, :DH] = 1.0
    sel2[0, 1, DH:] = 1.0
    sel2 = sel2.astype(BF)

    # per-batch feature tables (evaluated at every position)
    kfeat_b, phi_b, psi_b = [], [], []
    for b in range(B):
        fb = frac[b]
        pf, sf = _ev(phi, fb), _ev(psi, fb)
        phi_b.append(pf)
        psi_b.append(sf)
        kfeat_b.append(np.concatenate([pf, sf], 0).astype(BF))

    xqT_b = [
        np.ascontiguousarray(np.asarray(inputs["query"])[b].T).astype(BF)
        for b in range(B)
    ]
    xkT_b = [
        np.ascontiguousarray(np.asarray(inputs["key"])[b].T).astype(BF)
        for b in range(B)
    ]
    xvT_b = [
        np.ascontiguousarray(np.asarray(inputs["value"])[b].T).astype(BF)
        for b in range(B)
    ]

    in_maps = []
    for c in range(NCORES):
        b, hg = c // 4, c % 4
        h0 = HL * hg  # first global head

        wvP = np.zeros((D, HL * VG), np.float64)
        bvA = np.zeros((1, HL * VG), np.float64)
        for j in range(HL):
            wvP[:, VG * j : VG * j + DH] = wvT_full[:, DH * (h0 + j) : DH * (h0 + j) + DH]
            bvA[0, VG * j : VG * j + DH] = bv_full[DH * (h0 + j) : DH * (h0 + j) + DH]
            bvA[0, VG * j + DH] = 1.0

        qfeat = np.zeros((HL * AUG, T), np.float64)
        for j in range(HL):
            h = h0 + j
            a_h = delta * ap_[h] / NGRID
            b_h = -delta * an_[h] / NGRID
            qfeat[AUG * j : AUG * j + R] = a_h * psi_b[b]
            qfeat[AUG * j + R : AUG * (j + 1)] = b_h * phi_b[b]

        in_maps.append(
            {
                "xqT": xqT_b[b],
                "xkT": xkT_b[b],
                "xvT": xvT_b[b],
                "wqT": np.ascontiguousarray(
                    wqT_full[:, DH * h0 : DH * h0 + HL * DH]
                ).astype(BF),
                "wkT": np.ascontiguousarray(
                    wkT_full[:, DH * h0 : DH * h0 + HL * DH]
                ).astype(BF),
                "wvP": wvP.astype(BF),
                "woP": np.ascontiguousarray(
                    woT_full[DH * h0 : DH * h0 + HL * DH, :]
                ),
                "bq": np.ascontiguousarray(bq_full[DH * h0 : DH * h0 + HL * DH])[
                    :, None
                ],
                "bk": np.ascontiguousarray(bk_full[DH * h0 : DH * h0 + HL * DH])[
                    :, None
                ],
                "bvA": bvA.astype(BF),
                "sel2": sel2,
                "bo": bo_full if hg == 0 else np.zeros_like(bo_full),
                "kfeat": kfeat_b[b],
                "qfeat": qfeat.astype(BF),
            }
        )
    return in_maps


def _run(inputs, trace=False, **kw):
    from concourse.bass_utils import run_bass_kernel_spmd

    bias_free = not np.any(np.asarray(inputs["bo"]))
    nc = _build(bias_free=bias_free)
    in_maps = _make_in_maps(inputs)
    res = run_bass_kernel_spmd(
        nc, in_maps, core_ids=list(range(NCORES)), trace=trace, **kw
    )
    out = np.zeros((B, T, D), np.float32)
    for c in range(NCORES):
        b, p = c // 4, c % 4
        o = np.asarray(res.results[c]["out"], np.float32)  # [QC*128, D]
        for qc in range(QC):
            out[b, TQ * qc + 128 * p : TQ * qc + 128 * p + 128] = o[
                128 * qc : 128 * qc + 128
            ]
    return out, res


def kernel(**inputs):
    out, _ = _run(inputs)
    return out


# revision 48
# speedup vs baseline: 1.1707x; 1.0630x over previous
"""Distributed Trainium2 kernel for CustomMultiHeadAttentionStoich.

Head-parallel sharding (8 cores): core c = (batch b=c//4, head-group hg=c%4);
each core owns heads [4*hg, 4*hg+4) of its batch and runs them over ALL 2048
queries.  There is NO pre-attention collective: Q/K/V projections for the
local heads are computed from replicated inputs, so the only communication is
a per-query-chunk ReduceScatter of the (row-parallel) output-projection
partials, pipelined behind the next chunk's attention compute.

The stoichiometric bias alpha_pos*relu(clamp(fk-fq)) + alpha_neg*min(clamp,0)
is a rank-2x16 SVD feature expansion of the piecewise-linear kernel of
(fk - fq), injected as extra contraction rows of the scores matmul
(head_dim 64 leaves idle PE rows: zero extra PE cycles).

Scores are computed in [key, query] layout; softmax denominators come from a
fused ones-column of V through the AV matmul; normalization uses a rank-1 PE
broadcast of the reciprocals and a DVE multiply straight out of PSUM.
"""

import sys

sys.path.insert(0, "/opt/trn_rl_repo")

import numpy as np
import ml_dtypes

BF = ml_dtypes.bfloat16

B, T, D, H, DH = 2, 2048, 1024, 16, 64
NCORES = 8
HL = 4  # local heads per core
R = 16  # SVD rank per clamp-kernel half
AUG = 2 * R
CP = DH + AUG  # contraction rows for the scores matmul
NGRID = 1024  # SVD grid
KC = T // 128  # 16 key chunks
VG = DH + 1  # V columns per head incl. ones column
QC = 4  # query chunks (ReduceScatter granularity)
TQ = T // QC  # 512 queries per chunk

_state = {}


def _features():
    """Rank-R SVD features of rc(x,y)=clip(x-y,0,0.2) on [0,1]^2."""
    if "grid" not in _state:
        g = (np.arange(NGRID) + 0.5) / NGRID
        M = np.clip(g[:, None] - g[None, :], 0.0, 0.2)
        U, S, Vt = np.linalg.svd(M, full_matrices=False)
        sc = np.sqrt(S[:R] * NGRID)
        _state["grid"] = g
        _state["phi"] = (U[:, :R] * sc).astype(np.float64)  # [NGRID, R] phi_j(x)
        _state["psi"] = (Vt[:R].T * sc).astype(np.float64)  # [NGRID, R] psi_j(y)
    return _state["grid"], _state["phi"], _state["psi"]


def _ev(tab, x):
    g = _state["grid"]
    return np.stack([np.interp(x, g, tab[:, j]) for j in range(R)])


def _build(bias_free=False):
    if "nc" in _state:
        return _state["nc"]

    import concourse.bass as bass
    import concourse.mybir as mybir
    import concourse.tile as tile
    from concourse import bacc

    dt = mybir.dt
    ts = bass.ts
    ds = bass.ds

    nc = bacc.Bacc(
        "TRN2",
        target_bir_lowering=False,
        debug=False,
        num_devices=NCORES,
    )

    # ---- kernel I/O (per-core shards; host pre-slices) ----
    xqT = nc.dram_tensor("xqT", [D, T], dt.bfloat16, kind="ExternalInput").ap()
    xkT = nc.dram_tensor("xkT", [D, T], dt.bfloat16, kind="ExternalInput").ap()
    xvT = nc.dram_tensor("xvT", [D, T], dt.bfloat16, kind="ExternalInput").ap()
    wqT = nc.dram_tensor("wqT", [D, HL * DH], dt.bfloat16, kind="ExternalInput").ap()
    wkT = nc.dram_tensor("wkT", [D, HL * DH], dt.bfloat16, kind="ExternalInput").ap()
    wvP = nc.dram_tensor("wvP", [D, HL * VG], dt.bfloat16, kind="ExternalInput").ap()
    woP = nc.dram_tensor("woP", [HL * DH, D], dt.bfloat16, kind="ExternalInput").ap()
    bqE = nc.dram_tensor("bq", [HL * DH, 1], dt.float32, kind="ExternalInput").ap()
    bkE = nc.dram_tensor("bk", [HL * DH, 1], dt.float32, kind="ExternalInput").ap()
    bvA = nc.dram_tensor("bvA", [1, HL * VG], dt.bfloat16, kind="ExternalInput").ap()
    boE = nc.dram_tensor("bo", [1, D], dt.bfloat16, kind="ExternalInput").ap()
    selE = nc.dram_tensor("sel2", [1, 2, 128], dt.bfloat16, kind="ExternalInput").ap()
    kfE = nc.dram_tensor("kfeat", [AUG, T], dt.bfloat16, kind="ExternalInput").ap()
    qfE = nc.dram_tensor("qfeat", [HL * AUG, T], dt.bfloat16, kind="ExternalInput").ap()
    outE = nc.dram_tensor("out", [QC * 128, D], dt.bfloat16, kind="ExternalOutput").ap()

    Exp = mybir.ActivationFunctionType.Exp
    Copy = mybir.ActivationFunctionType.Copy
    RG = [[0, 1, 2, 3], [4, 5, 6, 7]]
    Add = mybir.AluOpType.add

    with tile.TileContext(nc) as tc:
        with (
            tc.tile_pool(name="dram", bufs=1, space="DRAM") as dram,
            tc.tile_pool(name="consts", bufs=1) as consts,
            tc.tile_pool(name="ehat", bufs=3) as ep,
            tc.tile_pool(name="dsum", bufs=6) as dsp,
            tc.tile_pool(name="rsb", bufs=2) as rsbp,
            tc.tile_pool(name="aot", bufs=4) as aotp,
            tc.tile_pool(name="ystage", bufs=4) as ysp,
            tc.tile_pool(name="pp", bufs=2, space="PSUM") as pp,
            tc.tile_pool(name="psS", bufs=2, space="PSUM") as psS,
            tc.tile_pool(name="psAV", bufs=2, space="PSUM") as psAV,
        ):
            rs_in = [
                dram.tile([TQ, D], dt.bfloat16, tag=f"rs_in{q}", name=f"rs_in{q}")
                for q in range(QC)
            ]
            rs_out = [
                dram.tile([128, D], dt.bfloat16, tag=f"rs_out{q}", name=f"rs_out{q}")
                for q in range(QC)
            ]

            # ---- resident constants / activations ----
            xq_sb = consts.tile([128, 8, T], dt.bfloat16, tag="xq", name="xq")
            xk_sb = consts.tile([128, 8, T], dt.bfloat16, tag="xk", name="xk")
            xv_sb = consts.tile([128, 8, T], dt.bfloat16, tag="xv", name="xv")
            wq_sb = consts.tile([128, 8, HL * DH], dt.bfloat16, tag="wq", name="wq")
            wk_sb = consts.tile([128, 8, HL * DH], dt.bfloat16, tag="wk", name="wk")
            wv_sb = consts.tile([128, 8, HL * VG], dt.bfloat16, tag="wv", name="wv")
            wo_sb = consts.tile([128, 2, D], dt.bfloat16, tag="wo", name="wo")
            bq_sb = consts.tile([128, 2, 1], dt.float32, tag="bq", name="bq")
            bk_sb = consts.tile([128, 2, 1], dt.float32, tag="bk", name="bk")
            bv_sb = consts.tile([1, HL * VG], dt.bfloat16, tag="bv", name="bv")
            bo_sb = consts.tile([1, D], dt.bfloat16, tag="bo", name="bo")
            ones_sb = consts.tile([1, 128], dt.bfloat16, tag="ones", name="ones")
            sel2 = consts.tile([1, 2, 128], dt.bfloat16, tag="sel2", name="sel2")

            kat = [
                consts.tile([CP, T], dt.bfloat16, tag=f"kat{h}", name=f"kat{h}")
                for h in range(HL)
            ]
            qat = [
                consts.tile([CP, T], dt.bfloat16, tag=f"qat{h}", name=f"qat{h}")
                for h in range(HL)
            ]
            vtl = consts.tile([128, KC, HL * VG], dt.bfloat16, tag="vtl", name="vtl")

            # ---- DMA loads (weights first, then per-contraction-chunk x) ----
            nc.sync.dma_start(
                out=wk_sb, in_=wkT.rearrange("(a p) m -> p a m", p=128)
            )
            nc.sync.dma_start(out=bk_sb, in_=bkE.rearrange("(g p) o -> p g o", p=128))
            for a in range(8):
                nc.sync.dma_start(out=xk_sb[:, a, :], in_=xkT[ds(128 * a, 128), :])
            nc.sync.dma_start(
                out=wv_sb, in_=wvP.rearrange("(a p) m -> p a m", p=128)
            )
            nc.sync.dma_start(out=bv_sb, in_=bvA)
            for a in range(8):
                nc.sync.dma_start(out=xv_sb[:, a, :], in_=xvT[ds(128 * a, 128), :])
            nc.sync.dma_start(
                out=wq_sb, in_=wqT.rearrange("(a p) m -> p a m", p=128)
            )
            nc.sync.dma_start(out=bq_sb, in_=bqE.rearrange("(g p) o -> p g o", p=128))
            for a in range(8):
                nc.sync.dma_start(out=xq_sb[:, a, :], in_=xqT[ds(128 * a, 128), :])
            nc.sync.dma_start(out=wo_sb, in_=woP.rearrange("(g p) m -> p g m", p=128))
            nc.sync.dma_start(out=bo_sb, in_=boE)
            for h in range(HL):
                nc.sync.dma_start(out=kat[h][DH:CP, :], in_=kfE)
                nc.sync.dma_start(out=qat[h][DH:CP, :], in_=qfE[ds(AUG * h, AUG), :])
            nc.vector.memset(ones_sb, 1.0)
            nc.sync.dma_start(out=sel2, in_=selE)

            # ---- warmup collective: absorb first-RS setup cost early ----
            wu_in = dram.tile([4, 128], dt.bfloat16, tag="wu_in", name="wu_in")
            wu_out = dram.tile([1, 128], dt.bfloat16, tag="wu_out", name="wu_out")
            wu_sb = consts.tile([4, 128], dt.bfloat16, tag="wu_sb", name="wu_sb")
            nc.vector.memset(wu_sb, 0.0)
            nc.sync.dma_start(out=wu_in, in_=wu_sb)
            nc.gpsimd.collective_compute(
                "ReduceScatter",
                Add,
                ins=[wu_in.opt()],
                outs=[wu_out.opt()],
                replica_groups=RG,
            )


            # ---- K^T projection: kat[h][0:64, :] = Wk_h @ xk^T + bk ----
            for g in range(2):
                for tc_i in range(4):
                    ps = pp.tile([128, 512], dt.float32, tag="mm", name="mmk")
                    for kc in range(8):
                        nc.tensor.matmul(
                            ps,
                            lhsT=wk_sb[:, kc, ts(g, 128)],
                            rhs=xk_sb[:, kc, ts(tc_i, 512)],
                            start=(kc == 0),
                            stop=(kc == 7),
                        )
                    for j in range(2):
                        nc.vector.tensor_scalar_add(
                            kat[2 * g + j][0:DH, ts(tc_i, 512)],
                            ps[ds(DH * j, DH), :],
                            bk_sb[ds(DH * j, DH), g, :],
                        )

            # ---- V projection (ones-column fused): vtl[:, t, :] ----
            for tc_i in range(KC):
                ps = pp.tile([128, HL * VG], dt.float32, tag="mm", name="mmv")
                for kc in range(8):
                    nc.tensor.matmul(
                        ps,
                        lhsT=xv_sb[:, kc, ts(tc_i, 128)],
                        rhs=wv_sb[:, kc, :],
                        start=(kc == 0),
                        stop=False,
                    )
                nc.tensor.matmul(
                    ps,
                    lhsT=ones_sb[:, :],
                    rhs=bv_sb[:, :],
                    start=False,
                    stop=True,
                )
                nc.vector.tensor_copy(vtl[:, tc_i, :], ps)

            # ---- Q projection: qat[h][0:64, :] = gs * Wq_h @ xq^T + bq ----
            for g in range(2):
                for tc_i in range(4):
                    ps = pp.tile([128, 512], dt.float32, tag="mm", name="mmq")
                    for kc in range(8):
                        nc.tensor.matmul(
                            ps,
                            lhsT=wq_sb[:, kc, ts(g, 128)],
                            rhs=xq_sb[:, kc, ts(tc_i, 512)],
                            start=(kc == 0),
                            stop=(kc == 7),
                        )
                    for j in range(2):
                        nc.vector.tensor_scalar_add(
                            qat[2 * g + j][0:DH, ts(tc_i, 512)],
                            ps[ds(DH * j, DH), :],
                            bq_sb[ds(DH * j, DH), g, :],
                        )

            # ---- attention + row-parallel out-projection, per query chunk ----
            # The PE queue is in-order, so AV(kp) is emitted AFTER S(kp+1):
            # the PE streams scores while the scalar engine exps the previous
            # pair of key chunks, and AV picks up the exp'd tile one step
            # later. Scalar runs a pure-exp stream (all small copies on DVE).
            KP = KC // 2

            def emit_outproj_group(pqc, p_aot, qs, mc):
                ps_y = pp.tile([128, 512], dt.float32, tag="mm", name="mmy")
                for g in range(2):
                    nc.tensor.matmul(
                        ps_y,
                        lhsT=p_aot[g][:, ts(qs, 128)],
                        rhs=wo_sb[:, g, ts(mc, 512)],
                        start=(g == 0),
                        stop=(g == 1 and bias_free),
                    )
                if not bias_free:
                    nc.tensor.matmul(
                        ps_y,
                        lhsT=ones_sb,
                        rhs=bo_sb[:, ts(mc, 512)],
                        start=False,
                        stop=True,
                    )
                ystg = ysp.tile([128, 512], dt.bfloat16, tag="ystage", name="ystage")
                nc.vector.tensor_copy(ystg, ps_y)
                nc.sync.dma_start(
                    out=rs_in[pqc][ds(128 * qs, 128), ts(mc, 512)],
                    in_=ystg,
                )

            def emit_collective(pqc):
                nc.gpsimd.collective_compute(
                    "ReduceScatter",
                    Add,
                    ins=[rs_in[pqc].opt()],
                    outs=[rs_out[pqc].opt()],
                    replica_groups=RG,
                )

            for qc in range(QC):
                aot_pair = [
                    aotp.tile([128, TQ], dt.bfloat16, tag=f"aot{g}", name=f"aot{g}")
                    for g in range(2)
                ]
                av_by = {}
                rbf_by = {}
                pending = []

                def flush_av():
                    h, kp, eh, ps_av = pending.pop(0)
                    for u in range(2):
                        nc.tensor.matmul(
                            ps_av,
                            lhsT=vtl[:, 2 * kp + u, ds(VG * h, VG)],
                            rhs=eh[:, u, :],
                            start=(kp == 0 and u == 0),
                            stop=(kp == KP - 1 and u == 1),
                        )
                    if kp == KP - 1:
                        dsum = dsp.tile([1, TQ], dt.float32, tag="dsum", name="dsum")
                        rcp = dsp.tile([1, TQ], dt.float32, tag="rcp", name="rcp")
                        rbf = dsp.tile(
                            [1, TQ], dt.bfloat16, tag="rcpbf", name="rcpbf"
                        )
                        nc.vector.tensor_copy(dsum, ps_av[ds(DH, 1), :])
                        nc.vector.reciprocal_approx_fast(rcp, dsum)
                        nc.vector.tensor_copy(rbf, rcp)
                        rbf_by[h] = rbf

                def norm_pair(g):
                    ps_r = pp.tile([128, TQ], dt.float32, tag="mm", name="mmr")
                    for j in range(2):
                        nc.tensor.matmul(
                            ps_r,
                            lhsT=sel2[:, j, :],
                            rhs=rbf_by[2 * g + j],
                            start=(j == 0),
                            stop=(j == 1),
                        )
                    r_sb = rsbp.tile([128, TQ], dt.bfloat16, tag="rsb", name="rsb")
                    nc.vector.tensor_copy(r_sb, ps_r)
                    for j in range(2):
                        nc.vector.tensor_mul(
                            aot_pair[g][ds(DH * j, DH), :],
                            av_by[2 * g + j][0:DH, :],
                            r_sb[ds(DH * j, DH), :],
                        )

                norm_due = None
                for h in range(HL):
                    ps_av = psAV.tile([VG, TQ], dt.float32, tag="av", name="av")
                    av_by[h] = ps_av
                    for kp in range(KP):
                        ps_s = psS.tile([128, 2, TQ], dt.float32, tag="s", name="s")
                        eh = ep.tile([128, 2, TQ], dt.bfloat16, tag="ehat", name="eh")
                        for u in range(2):
                            nc.tensor.matmul(
                                ps_s[:, u, :],
                                lhsT=kat[h][:, ts(2 * kp + u, 128)],
                                rhs=qat[h][:, ts(qc, TQ)],
                                start=True,
                                stop=True,
                            )
                        nc.scalar.activation(eh, ps_s, Exp)
                        if norm_due is not None:
                            # normalization of the finished pair goes into the
                            # PE queue BEFORE this head's first AV (which
                            # recycles that pair's PSUM buffer)
                            norm_pair(norm_due)
                            norm_due = None
                        if pending:
                            flush_av()
                        pending.append((h, kp, eh, ps_av))
                    if h % 2 == 1:
                        while pending:
                            flush_av()
                        norm_due = h // 2
                if norm_due is not None:
                    norm_pair(norm_due)
                    norm_due = None
                for qs in range(4):
                    for mc in range(2):
                        emit_outproj_group(qc, aot_pair, qs, mc)
                emit_collective(qc)

            for qc in range(QC):
                nc.sync.dma_start(
                    out=outE[ds(128 * qc, 128), :], in_=rs_out[qc][:, :]
                )

    nc.compile()
    _state["nc"] = nc
    return nc


def _make_in_maps(inputs):
    _features()
    gs = float(np.float32(inputs["gamma"])) * DH ** -0.5
    delta = float(np.float32(inputs["delta"]))
    ap_ = np.asarray(inputs["alpha_pos"], np.float64)
    an_ = np.asarray(inputs["alpha_neg"], np.float64)

    wqT_full = np.asarray(inputs["Wq"], np.float64).T * gs
    bq_full = (np.asarray(inputs["bq"], np.float64) * gs).astype(np.float32)
    wkT_full = np.ascontiguousarray(np.asarray(inputs["Wk"]).T)
    bk_full = np.asarray(inputs["bk"], np.float32)
    wvT_full = np.asarray(inputs["Wv"], np.float64).T
    bv_full = np.asarray(inputs["bv"], np.float64)
    woT_full = np.ascontiguousarray(np.asarray(inputs["Wo"]).T).astype(BF)
    bo_full = np.asarray(inputs["bo"], np.float32)[None, :].astype(BF)

    phi, psi = _state["phi"], _state["psi"]
    frac = np.asarray(inputs["frac"], np.float64)

    sel2 = np.zeros((1, 2, 128), np.float64)
    sel2[0, 0, :DH] = 1.0
    sel2[0, 1, DH:] = 1.0
    sel2 = sel2.astype(BF)

    # per-batch feature tables (evaluated at every position)
    kfeat_b, phi_b, psi_b = [], [], []
    for b in range(B):
        fb = frac[b]
        pf, sf = _ev(phi, fb), _ev(psi, fb)
        phi_b.append(pf)
        psi_b.append(sf)
        kfeat_b.append(np.concatenate([pf, sf], 0).astype(BF))

    xqT_b = [
        np.ascontiguousarray(np.asarray(inputs["query"])[b].T).astype(BF)
        for b in range(B)
    ]
    xkT_b = [
        np.ascontiguousarray(np.asarray(inputs["key"])[b].T).astype(BF)
        for b in range(B)
    ]
    xvT_b = [
        np.ascontiguousarray(np.asarray(inputs["value"])[b].T).astype(BF)
        for b in range(B)
    ]

    in_maps = []
    for c in range(NCORES):
        b, hg = c // 4, c % 4
        h0 = HL * hg  # first global head

        wvP = np.zeros((D, HL * VG), np.float64)
        bvA = np.zeros((1, HL * VG), np.float64)
        for j in range(HL):
            wvP[:, VG * j : VG * j + DH] = wvT_full[:, DH * (h0 + j) : DH * (h0 + j) + DH]
            bvA[0, VG * j : VG * j + DH] = bv_full[DH * (h0 + j) : DH * (h0 + j) + DH]
            bvA[0, VG * j + DH] = 1.0

        qfeat = np.zeros((HL * AUG, T), np.float64)
        for j in range(HL):
            h = h0 + j
            a_h = delta * ap_[h] / NGRID
            b_h = -delta * an_[h] / NGRID
            qfeat[AUG * j : AUG * j + R] = a_h * psi_b[b]
            qfeat[AUG * j + R : AUG * (j + 1)] = b_h * phi_b[b]

        in_maps.append(
            {
                "xqT": xqT_b[b],
                "xkT": xkT_b[b],
                "xvT": xvT_b[b],
                "wqT": np.ascontiguousarray(
                    wqT_full[:, DH * h0 : DH * h0 + HL * DH]
                ).astype(BF),
                "wkT": np.ascontiguousarray(
                    wkT_full[:, DH * h0 : DH * h0 + HL * DH]
                ).astype(BF),
                "wvP": wvP.astype(BF),
                "woP": np.ascontiguousarray(
                    woT_full[DH * h0 : DH * h0 + HL * DH, :]
                ),
                "bq": np.ascontiguousarray(bq_full[DH * h0 : DH * h0 + HL * DH])[
                    :, None
                ],
                "bk": np.ascontiguousarray(bk_full[DH * h0 : DH * h0 + HL * DH])[
                    :, None
                ],
                "bvA": bvA.astype(BF),
                "sel2": sel2,
                "bo": bo_full if hg == 0 else np.zeros_like(bo_full),
                "kfeat": kfeat_b[b],
                "qfeat": qfeat.astype(BF),
            }
        )
    return in_maps


def _run(inputs, trace=False, **kw):
    from concourse.bass_utils import run_bass_kernel_spmd

    bias_free = not np.any(np.asarray(inputs["bo"]))
    nc = _build(bias_free=bias_free)
    in_maps = _make_in_maps(inputs)
    res = run_bass_kernel_spmd(
        nc, in_maps, core_ids=list(range(NCORES)), trace=trace, **kw
    )
    out = np.zeros((B, T, D), np.float32)
    for c in range(NCORES):
        b, p = c // 4, c % 4
        o = np.asarray(res.results[c]["out"], np.float32)  # [QC*128, D]
        for qc in range(QC):
            out[b, TQ * qc + 128 * p : TQ * qc + 128 * p + 128] = o[
                128 * qc : 128 * qc + 128
            ]
    return out, res


def kernel(**inputs):
    out, _ = _run(inputs)
    return out
